# revision 17
# baseline (speedup 1.0000x reference)
"""Trainium2 Bass kernel for nn_BreakthroughSNN (spiking SSM LM).

Strategy (8 NeuronCores, SPMD single NEFF, fully independent cores):
  - Data-parallel SSM: 2048 tokens (B*S) sharded 256/core. Per core, the
    4-layer x 20-step LIF recurrence runs with persistent membrane
    potentials held in PSUM.
  - All SSM matmuls are fp32r hi/lo pairs (host-split so the device's
    fp32r rounding is exact) -> full fp32-grade precision at 1 cyc/row.
  - Temporal encoding via host-precomputed exact fp32 sigmoid-boundary
    thresholds: per-chunk threshold-count index built with fused
    is_ge/add chains on DVE; the one-hot spike planes are materialized
    just-in-time inside layer 0's step loop (is_equal per step).
  - LIF elementwise work spread across ACT (Sign from PSUM, spike
    writes), GPSIMD (mask affine derivations in SBUF) and DVE (PSUM
    multiply updates) so no single engine gates the recurrence.
  - Projection: token-sharded - each core projects its OWN 256 tokens
    against the FULL 32000 vocab. Wp is streamed as bf16 (pre-scaled by
    1/T on host so the time-integrated spike counts stay integer-exact),
    double-buffered so DMA overlaps the matmuls. No collective at all.
    Output is written bf16 (post-chaos linear op) and upcast + bias on
    host.
"""

import numpy as np
import ml_dtypes
from contextlib import ExitStack

import concourse.bass as bass
import concourse.mybir as mybir
import concourse.tile as tile
from concourse import bacc
from concourse.bass_utils import run_bass_kernel_spmd
from concourse.masks import make_identity

F32 = mybir.dt.float32
F32R = mybir.dt.float32r
BF16 = mybir.dt.bfloat16
I32 = mybir.dt.int32
OP = mybir.AluOpType
ACTF = mybir.ActivationFunctionType

NCORES = 8
TOKPC = 256          # tokens per core
BATCH, SEQ = 4, 512
DM, DS = 512, 128
T, L = 20, 4
VOC = 32000
KC = DM // 128       # 4 feature chunks
NVW = 2000           # vocab cols per proj weight tile (4 psum banks of 500)
NVG = VOC // NVW     # 16 vocab groups
NV = 500             # one PSUM bank of fp32


def _hilo(x):
    x = np.ascontiguousarray(x, dtype=np.float32)
    u = x.view(np.uint32)
    hi = (u & np.uint32(0xFFFFF000)).view(np.float32).copy()  # keep 11 mantissa bits
    lo = (x - hi).astype(np.float32)
    return hi, lo


def _f2key(x):
    u = int(np.array(x, dtype=np.float32).view(np.uint32))
    return (u ^ 0x80000000) if u < 0x80000000 else (0xFFFFFFFF - u)


def _key2f(k):
    u = (k ^ 0x80000000) if k >= 0x80000000 else (0xFFFFFFFF - k)
    return np.array([u], dtype=np.uint32).view(np.float32)[0]


def _g32(x):
    # replicate reference fp32 pipeline: floor happens on this value
    x = np.float32(x)
    s = np.float32(1.0) / (np.float32(1.0) + np.float32(np.exp(np.float32(-x))))
    return np.float32(s * np.float32(19.0))


def _thresholds():
    """T_k = smallest fp32 x with g32(x) >= k, k=1..19 (g32 monotone)."""
    ts = []
    for k in range(1, 20):
        lo_k = _f2key(np.float32(-30.0))
        hi_k = _f2key(np.float32(30.0))
        assert _g32(_key2f(hi_k)) >= k and _g32(_key2f(lo_k)) < k
        while hi_k - lo_k > 1:
            mid = (lo_k + hi_k) // 2
            if _g32(_key2f(mid)) >= k:
                hi_k = mid
            else:
                lo_k = mid
        ts.append(float(_key2f(hi_k)))
    return ts


def _build_nc():
    nc = bacc.Bacc("TRN2", target_bir_lowering=False, debug=False, num_devices=NCORES)

    ids_d = nc.dram_tensor("ids", [2, 128, 1], I32, kind="ExternalInput")
    emb_d = nc.dram_tensor("emb", [VOC, DM], F32, kind="ExternalInput")
    at_hi_d = nc.dram_tensor("at_hi", [L, 128, 128], F32R, kind="ExternalInput")
    at_lo_d = nc.dram_tensor("at_lo", [L, 128, 128], F32R, kind="ExternalInput")
    bt_hi_d = nc.dram_tensor("bt_hi", [L, 128, KC, 128], F32R, kind="ExternalInput")
    bt_lo_d = nc.dram_tensor("bt_lo", [L, 128, KC, 128], F32R, kind="ExternalInput")
    ct_hi_d = nc.dram_tensor("ct_hi", [L, 128, KC, 128], F32R, kind="ExternalInput")
    ct_lo_d = nc.dram_tensor("ct_lo", [L, 128, KC, 128], F32R, kind="ExternalInput")
    dd_hi_d = nc.dram_tensor("dd_hi", [L, 128, KC, 128], F32R, kind="ExternalInput")
    dd_lo_d = nc.dram_tensor("dd_lo", [L, 128, KC, 128], F32R, kind="ExternalInput")
    wpt_d = nc.dram_tensor("wpt", [DM, VOC], BF16, kind="ExternalInput")
    out_d = nc.dram_tensor("out", [TOKPC, VOC], BF16, kind="ExternalOutput")

    THR = _thresholds()

    with tile.TileContext(nc) as tc, ExitStack() as ctx:
        const = ctx.enter_context(tc.tile_pool(name="const", bufs=1))
        ident = const.tile([128, 128], F32)
        make_identity(nc, ident[:])
        ident_r = const.tile([128, 128], F32R)
        nc.vector.tensor_copy(ident_r[:], ident[:])
        neg2 = const.tile([128, 1], F32)
        nc.vector.memset(neg2[:], -2.0)

        xb_pool = ctx.enter_context(tc.tile_pool(name="xb", bufs=1))
        xb = xb_pool.tile([128, T * KC * 256], F32R)
        tip = ctx.enter_context(tc.tile_pool(name="ti", bufs=1))
        ti_bf = tip.tile([128, KC * 256], BF16, tag="tibf")
        # projection weight pool created BEFORE the SSM pools so its SBUF
        # region doesn't overlap theirs -> the first weight-group DMAs can
        # prefetch during the SSM phase
        prw = ctx.enter_context(tc.tile_pool(name="prw", bufs=2))
        osbp = ctx.enter_context(tc.tile_pool(name="osb", bufs=3))

        # ---------------- encode: gather + transpose + sign one-hot --------
        # Compare ALU ops are pathologically slow on DVE/GPSIMD; instead
        # build step functions SG_t = Sign(EMB - T_t) in {-1,1} on ACT
        # (IEEE subtract preserves the >= boundary exactly) and difference
        # them with fast DVE subtracts: xb[t] = SG_t - SG_{t+1} in {0,2}.
        # The x2 scale is compensated by halving layer 0's B and D on host.
        emb4 = ctx.enter_context(tc.tile_pool(name="emb4", bufs=1))
        EMBC = emb4.tile([128, KC * TOKPC], F32, tag="embc")
        thr_b = []
        for j, tj in enumerate(THR):
            bt_ = const.tile([128, 1], F32, tag=f"thr{j}", name=f"thr{j}")
            nc.vector.memset(bt_[:], -float(tj))
            thr_b.append(bt_)
        sgp = ctx.enter_context(tc.tile_pool(name="sg", bufs=2))
        with tc.tile_pool(name="enc", bufs=2) as enc, \
             tc.tile_pool(name="encp", bufs=2, space="PSUM") as encps:
            ids_s = enc.tile([128, 2], I32, tag="ids")
            for g in range(2):
                nc.sync.dma_start(ids_s[:, g:g + 1], ids_d[g, :, :])
            for g in range(2):
                eg = enc.tile([128, DM], F32, tag="eg")
                nc.gpsimd.indirect_dma_start(
                    out=eg[:], out_offset=None,
                    in_=emb_d[:, :],
                    in_offset=bass.IndirectOffsetOnAxis(ap=ids_s[:, g:g + 1], axis=0),
                )
                for k in range(KC):
                    pt = encps.tile([128, 128], F32, tag="pt")
                    nc.tensor.transpose(pt[:], eg[:, k * 128:(k + 1) * 128], ident[:])
                    nc.scalar.copy(EMBC[:, k * 256 + g * 128:
                                        k * 256 + g * 128 + 128], pt[:])
            # ascending production, interleaved into layer 0's step loop
            # via emit_encode(t) so the in-order DVE queue never makes the
            # recurrence wait behind the whole encode tail
            _enc_state = {}

            def emit_encode(t):
                # produce xb[t]; requires emit_encode called in ascending t
                if t >= T:
                    return
                if t == 0:
                    sg = sgp.tile([128, KC * TOKPC], F32, tag="sg")
                    nc.scalar.activation(sg[:], EMBC[:], ACTF.Sign,
                                         bias=thr_b[0][:], scale=1.0)
                    nc.gpsimd.tensor_scalar(xb[:, 0:KC * 256], sg[:],
                                            -1.0, 1.0, OP.mult, OP.add)
                    _enc_state["sg"] = sg
                elif t < T - 1:
                    sg_new = sgp.tile([128, KC * TOKPC], F32, tag="sg")
                    nc.scalar.activation(sg_new[:], EMBC[:], ACTF.Sign,
                                         bias=thr_b[t][:], scale=1.0)
                    nc.gpsimd.tensor_tensor(
                        xb[:, t * KC * 256:(t + 1) * KC * 256],
                        _enc_state["sg"][:], sg_new[:], OP.subtract)
                    _enc_state["sg"] = sg_new
                else:
                    nc.gpsimd.tensor_scalar(
                        xb[:, (T - 1) * KC * 256:T * KC * 256],
                        _enc_state["sg"][:], 1.0, None, OP.add)

            emit_encode(0)
            emit_encode(1)

        # ---------------- SSM layers ---------------------------------------
        with tc.tile_pool(name="ssmp", bufs=1, space="PSUM") as ssmps, \
             tc.tile_pool(name="par", bufs=2) as par, \
             tc.tile_pool(name="lif", bufs=2) as lif:
            v1ps = ssmps.tile([128, TOKPC], F32, tag="v1")
            # v2 as two (128,512) tiles: pair j holds feature chunks 2j, 2j+1
            v2pr = [ssmps.tile([128, 2 * TOKPC], F32, tag=f"v2p{j}", name=f"v2pr{j}")
                    for j in range(2)]
            tips = ssmps.tile([128, KC * TOKPC], F32, tag="tips")

            Hprev = None
            for layer in range(L):
                def loadp(dram_ap, shape, tag):
                    pt_ = par.tile(list(shape), F32R, tag=tag, name=f"par_{tag}")
                    nc.sync.dma_start(pt_[:], dram_ap)
                    return pt_

                ah = loadp(at_hi_d[layer, :, :], (128, 128), "ah")
                al = loadp(at_lo_d[layer, :, :], (128, 128), "al")
                bh = loadp(bt_hi_d[layer, :, :, :], (128, KC, 128), "bh")
                bl = loadp(bt_lo_d[layer, :, :, :], (128, KC, 128), "bl")
                ch = loadp(ct_hi_d[layer, :, :, :], (128, KC, 128), "ch")
                cl = loadp(ct_lo_d[layer, :, :, :], (128, KC, 128), "cl")
                dh = loadp(dd_hi_d[layer, :, :, :], (128, KC, 128), "dh")
                dl = loadp(dd_lo_d[layer, :, :, :], (128, KC, 128), "dl")

                def emit_mm2_lif2(t, H_t, xs_t, layer_):
                    # output update accumulation (v2, per chunk) + LIF2
                    for k in range(KC):
                        vsl = v2pr[k // 2][:, (k % 2) * TOKPC:(k % 2 + 1) * TOKPC]
                        mm2 = [(ch[:, k, :], H_t[:]), (cl[:, k, :], H_t[:]),
                               (dh[:, k, :], xs_t[k]), (dl[:, k, :], xs_t[k])]
                        for i, (lhsT, rhs) in enumerate(mm2):
                            nc.tensor.matmul(vsl, lhsT, rhs,
                                             start=(t == 0 and i == 0 and k % 2 == 0),
                                             stop=(i == len(mm2) - 1),
                                             skip_group_check=True)
                    # LIF2: sg on ACT (psum read); bank-0 mask derived on DVE,
                    # bank-1 mask on GPSIMD, so the two 3-hop chains overlap;
                    # v2 *= mask and the spike write (f32r) stay on DVE
                    m2c = lif.tile([128, 2 * 2 * TOKPC], F32, tag="m2c")
                    for j in range(2):
                        sg2 = lif.tile([128, 2 * TOKPC], F32, tag=f"sg2_{j}",
                                       name=f"sg2_{j}")
                        nc.scalar.activation(sg2[:], v2pr[j][:], ACTF.Sign,
                                             bias=neg2[:], scale=1.0)
                        eng = nc.vector if j == 0 else nc.gpsimd
                        eng.tensor_scalar(m2c[:, j * 512:(j + 1) * 512],
                                          sg2[:], -0.25, 0.25,
                                          OP.mult, OP.add)
                        nc.vector.tensor_tensor(v2pr[j][:], v2pr[j][:],
                                                m2c[:, j * 512:(j + 1) * 512],
                                                OP.mult)
                    xsl = xb[:, t * KC * 256:(t + 1) * KC * 256]
                    nc.vector.tensor_scalar(xsl, m2c[:], -2.0, 1.0,
                                            OP.mult, OP.add)
                    if layer_ == L - 1:
                        # time-integration on the PE: tips += I @ X[t]
                        # (reads xsl AFTER the spike overwrite above)
                        for k in range(KC):
                            nc.tensor.matmul(
                                tips[:, k * TOKPC:(k + 1) * TOKPC],
                                ident_r[:], xs_t[k],
                                start=(t == 0 and k % 2 == 0),
                                stop=(t == T - 1),
                                skip_group_check=True)

                prev = None  # (t, H, xs) pending MM2+LIF2 (1-step software skew)
                for t in range(T):
                    if layer == 0:
                        emit_encode(t + 2)
                    xs = [xb[:, (t * KC + k) * 256:(t * KC + k) * 256 + 256]
                          for k in range(KC)]
                    # ---- state update accumulation (v1) ----
                    mm1 = []
                    if t > 0:
                        mm1 += [(ah[:], Hprev[:]), (al[:], Hprev[:])]
                    for k in range(KC):
                        mm1 += [(bh[:, k, :], xs[k]), (bl[:, k, :], xs[k])]
                    for i, (lhsT, rhs) in enumerate(mm1):
                        nc.tensor.matmul(v1ps[:], lhsT, rhs,
                                         start=(t == 0 and i == 0),
                                         stop=(i == len(mm1) - 1),
                                         skip_group_check=True)
                    # ---- LIF1: spike H straight from PSUM on DVE (f32r),
                    #      m1 derived off-path on GPSIMD ----
                    H = lif.tile([128, TOKPC], F32R, tag="H", bufs=3)
                    nc.vector.tensor_scalar(H[:], v1ps[:], 2.0, None, OP.is_ge)
                    m1 = lif.tile([128, TOKPC], F32, tag="m1")
                    nc.gpsimd.tensor_scalar(m1[:], H[:].bitcast(F32), -0.5, 0.5,
                                            OP.mult, OP.add)
                    nc.vector.tensor_tensor(v1ps[:], v1ps[:], m1[:], OP.mult)
                    # ---- previous step's output-side work (keeps PE fed) ----
                    if prev is not None:
                        emit_mm2_lif2(*prev, layer)
                    prev = (t, H, xs)
                    Hprev = H
                emit_mm2_lif2(*prev, layer)

            # time-integrated spike counts -> bf16 (exact integers 0..20;
            # the 1/T scale is folded into Wp on the host)
            for j in range(2):
                nc.scalar.activation(ti_bf[:, j * 512:(j + 1) * 512],
                                     tips[:, j * 512:(j + 1) * 512],
                                     ACTF.Copy, bias=0.0, scale=1.0)

        # ---------------- projection: own 256 tokens x full vocab ----------
        # weight-group DMA triggers are emitted one group AHEAD of the
        # consuming matmuls: the SP engine processes (and blocks on) DMA
        # triggers in order, so output DMAs must not sit in front of the
        # next group's prefetch. Output goes PSUM -> DRAM directly (f32).
        with tc.tile_pool(name="prjp", bufs=2, space="PSUM") as prjps:
            def fetch_w(g):
                wts = []
                for k in range(KC):
                    wt = prw.tile([128, NVW], BF16, tag=f"wt{k}", name=f"wt{k}")
                    eng = nc.sync if k < 2 else nc.scalar
                    eng.dma_start(wt[:], wpt_d[k * 128:(k + 1) * 128,
                                               g * NVW:(g + 1) * NVW])
                    wts.append(wt)
                return wts

            wts_next = fetch_w(0)
            for g in range(NVG):
                wts = wts_next
                if g + 1 < NVG:
                    wts_next = fetch_w(g + 1)
                for m in range(TOKPC // 128):
                    pos = [prjps.tile([128, NV], F32, tag=f"po{nv}",
                                      name=f"po{nv}") for nv in range(NVW // NV)]
                    for k in range(KC):
                        lh = ti_bf[:, k * 256 + m * 128: k * 256 + m * 128 + 128]
                        for nv in range(NVW // NV):
                            nc.tensor.matmul(pos[nv][:], lh,
                                             wts[k][:, nv * NV:(nv + 1) * NV],
                                             start=(k == 0), stop=(k == KC - 1),
                                             skip_group_check=True)
                    osb = osbp.tile([128, NVW], BF16, tag="osb")
                    for nv in range(NVW // NV):
                        nc.scalar.activation(osb[:, nv * NV:(nv + 1) * NV],
                                             pos[nv][:], ACTF.Copy,
                                             bias=0.0, scale=1.0)
                    nc.sync.dma_start(out_d[m * 128:(m + 1) * 128,
                                            g * NVW:(g + 1) * NVW], osb[:])

    nc.compile()
    return nc


_NC_CACHE = {}
_last_in_maps = None


def _get_nc():
    if "nc" not in _NC_CACHE:
        _NC_CACHE["nc"] = _build_nc()
    return _NC_CACHE["nc"]


def kernel(input_ids, emb_table, A, B, C, D, Wp, bp):
    input_ids = np.asarray(input_ids)
    emb_table = np.ascontiguousarray(np.asarray(emb_table), dtype=np.float32)
    A = np.asarray(A, dtype=np.float32)
    B = np.asarray(B, dtype=np.float32)
    C = np.asarray(C, dtype=np.float32)
    D = np.asarray(D, dtype=np.float32)
    Wp = np.asarray(Wp, dtype=np.float32)
    bp = np.asarray(bp, dtype=np.float32)

    ids_flat = input_ids.reshape(-1).astype(np.int32)          # (2048,)

    at = np.ascontiguousarray(A.transpose(0, 2, 1))            # (L,128,128)
    at_hi, at_lo = _hilo(at)
    Bh = B.copy()
    Bh[0] *= np.float32(0.5)   # layer-0 spikes arrive x2 scaled from encode
    bt = np.ascontiguousarray(
        Bh.transpose(2, 0, 1).reshape(KC, 128, L, DS).transpose(2, 1, 0, 3))
    # bt[l,p,k,m] = B[l, m, k*128+p]
    bt_hi, bt_lo = _hilo(bt)
    ct = np.ascontiguousarray(C.transpose(0, 2, 1).reshape(L, 128, KC, 128))
    # ct[l,p,mc,m] = C[l, mc*128+m, p]
    ct_hi, ct_lo = _hilo(ct)
    Dh = D.copy()
    Dh[0] *= np.float32(0.5)   # layer-0 spikes arrive x2 scaled from encode
    dc = Dh.reshape(L, KC, 128)                                # (L,KC,128)
    dc_hi, dc_lo = _hilo(dc)
    eye = np.eye(128, dtype=np.float32)
    # diag matrices for the D (elementwise) term, built on host
    dd_hi = np.ascontiguousarray(
        (eye[None, None, :, :] * dc_hi[:, :, None, :]).transpose(0, 2, 1, 3))
    dd_lo = np.ascontiguousarray(
        (eye[None, None, :, :] * dc_lo[:, :, None, :]).transpose(0, 2, 1, 3))
    # dd[l, p, k, m] = D[l, k*128+p] if p == m else 0

    wpt = np.ascontiguousarray(Wp.T) * np.float32(1.0 / T)     # (512, 32000) f32
    wpt_bf = wpt.astype(ml_dtypes.bfloat16)

    nc = _get_nc()
    in_maps = []
    for c in range(NCORES):
        ids_c = ids_flat[c * TOKPC:(c + 1) * TOKPC].reshape(2, 128, 1)
        in_maps.append({
            "ids": np.ascontiguousarray(ids_c),
            "emb": emb_table,
            "at_hi": at_hi, "at_lo": at_lo,
            "bt_hi": bt_hi, "bt_lo": bt_lo,
            "ct_hi": ct_hi, "ct_lo": ct_lo,
            "dd_hi": dd_hi, "dd_lo": dd_lo,
            "wpt": wpt_bf,
        })

    global _last_in_maps
    _last_in_maps = in_maps
    res = run_bass_kernel_spmd(nc, in_maps, core_ids=list(range(NCORES)))
    outs = [res.results[c]["out"].astype(np.float32) for c in range(NCORES)]
    full = np.concatenate(outs, axis=0)                        # (2048, 32000)
    full += bp[None, :]
    return full.reshape(BATCH, SEQ, VOC).astype(np.float32)


# revision 18
# speedup vs baseline: 1.2294x; 1.2294x over previous
"""Trainium2 Bass kernel for nn_BreakthroughSNN (spiking SSM LM).

Strategy (8 NeuronCores, SPMD single NEFF, fully independent cores):
  - Data-parallel SSM: 2048 tokens (B*S) sharded 256/core. Per core, the
    4-layer x 20-step LIF recurrence runs with persistent membrane
    potentials held in PSUM.
  - All SSM matmuls are fp32r hi/lo pairs (host-split so the device's
    fp32r rounding is exact) -> full fp32-grade precision at 1 cyc/row.
  - Temporal encoding via host-precomputed exact fp32 sigmoid-boundary
    thresholds: per-chunk threshold-count index built with fused
    is_ge/add chains on DVE; the one-hot spike planes are materialized
    just-in-time inside layer 0's step loop (is_equal per step).
  - LIF elementwise work spread across ACT (Sign from PSUM, spike
    writes), GPSIMD (mask affine derivations in SBUF) and DVE (PSUM
    multiply updates) so no single engine gates the recurrence.
  - Projection: token-sharded - each core projects its OWN 256 tokens
    against the FULL 32000 vocab. Wp is streamed as bf16 (pre-scaled by
    1/T on host so the time-integrated spike counts stay integer-exact),
    double-buffered so DMA overlaps the matmuls. No collective at all.
    Output is written bf16 (post-chaos linear op) and upcast + bias on
    host.
"""

import numpy as np
import ml_dtypes
from contextlib import ExitStack

import concourse.bass as bass
import concourse.mybir as mybir
import concourse.tile as tile
from concourse import bacc
from concourse.bass_utils import run_bass_kernel_spmd
from concourse.masks import make_identity

F32 = mybir.dt.float32
F32R = mybir.dt.float32r
BF16 = mybir.dt.bfloat16
I32 = mybir.dt.int32
OP = mybir.AluOpType
ACTF = mybir.ActivationFunctionType

NCORES = 8
TOKPC = 256          # tokens per core
BATCH, SEQ = 4, 512
DM, DS = 512, 128
T, L = 20, 4
VOC = 32000
KC = DM // 128       # 4 feature chunks
NVW = 2000           # vocab cols per proj weight tile (4 psum banks of 500)
NVG = VOC // NVW     # 16 vocab groups
NV = 500             # one PSUM bank of fp32


def _hilo(x):
    x = np.ascontiguousarray(x, dtype=np.float32)
    u = x.view(np.uint32)
    hi = (u & np.uint32(0xFFFFF000)).view(np.float32).copy()  # keep 11 mantissa bits
    lo = (x - hi).astype(np.float32)
    return hi, lo


def _f2key(x):
    u = int(np.array(x, dtype=np.float32).view(np.uint32))
    return (u ^ 0x80000000) if u < 0x80000000 else (0xFFFFFFFF - u)


def _key2f(k):
    u = (k ^ 0x80000000) if k >= 0x80000000 else (0xFFFFFFFF - k)
    return np.array([u], dtype=np.uint32).view(np.float32)[0]


def _g32(x):
    # replicate reference fp32 pipeline: floor happens on this value
    x = np.float32(x)
    s = np.float32(1.0) / (np.float32(1.0) + np.float32(np.exp(np.float32(-x))))
    return np.float32(s * np.float32(19.0))


def _thresholds():
    """T_k = smallest fp32 x with g32(x) >= k, k=1..19 (g32 monotone)."""
    ts = []
    for k in range(1, 20):
        lo_k = _f2key(np.float32(-30.0))
        hi_k = _f2key(np.float32(30.0))
        assert _g32(_key2f(hi_k)) >= k and _g32(_key2f(lo_k)) < k
        while hi_k - lo_k > 1:
            mid = (lo_k + hi_k) // 2
            if _g32(_key2f(mid)) >= k:
                hi_k = mid
            else:
                lo_k = mid
        ts.append(float(_key2f(hi_k)))
    return ts


def _build_nc():
    nc = bacc.Bacc("TRN2", target_bir_lowering=False, debug=False, num_devices=NCORES)

    ids_d = nc.dram_tensor("ids", [2, 128, 1], I32, kind="ExternalInput")
    emb_d = nc.dram_tensor("emb", [VOC, DM], F32, kind="ExternalInput")
    at_hi_d = nc.dram_tensor("at_hi", [L, 128, 128], F32R, kind="ExternalInput")
    at_lo_d = nc.dram_tensor("at_lo", [L, 128, 128], F32R, kind="ExternalInput")
    bt_hi_d = nc.dram_tensor("bt_hi", [L, 128, KC, 128], F32R, kind="ExternalInput")
    bt_lo_d = nc.dram_tensor("bt_lo", [L, 128, KC, 128], F32R, kind="ExternalInput")
    ct_hi_d = nc.dram_tensor("ct_hi", [L, 128, KC, 128], F32R, kind="ExternalInput")
    ct_lo_d = nc.dram_tensor("ct_lo", [L, 128, KC, 128], F32R, kind="ExternalInput")
    dd_hi_d = nc.dram_tensor("dd_hi", [L, 128, KC, 128], F32R, kind="ExternalInput")
    dd_lo_d = nc.dram_tensor("dd_lo", [L, 128, KC, 128], F32R, kind="ExternalInput")
    wpt_d = nc.dram_tensor("wpt", [DM, VOC], BF16, kind="ExternalInput")
    out_d = nc.dram_tensor("out", [TOKPC, VOC], BF16, kind="ExternalOutput")

    THR = _thresholds()

    with tile.TileContext(nc) as tc, ExitStack() as ctx:
        const = ctx.enter_context(tc.tile_pool(name="const", bufs=1))
        ident = const.tile([128, 128], F32)
        make_identity(nc, ident[:])
        ident_r = const.tile([128, 128], F32R)
        nc.vector.tensor_copy(ident_r[:], ident[:])
        neg2 = const.tile([128, 1], F32)
        nc.vector.memset(neg2[:], -2.0)

        xb_pool = ctx.enter_context(tc.tile_pool(name="xb", bufs=1))
        xb = xb_pool.tile([128, T * KC * 256], F32R)
        tip = ctx.enter_context(tc.tile_pool(name="ti", bufs=1))
        ti_bf = tip.tile([128, KC * 256], BF16, tag="tibf")
        # projection weight pool created BEFORE the SSM pools so its SBUF
        # region doesn't overlap theirs -> the first weight-group DMAs can
        # prefetch during the SSM phase
        prw = ctx.enter_context(tc.tile_pool(name="prw", bufs=2))
        osbp = ctx.enter_context(tc.tile_pool(name="osb", bufs=3))

        # ---------------- encode: gather + transpose + sign one-hot --------
        # Compare ALU ops are pathologically slow on DVE/GPSIMD; instead
        # build step functions SG_t = Sign(EMB - T_t) in {-1,1} on ACT
        # (IEEE subtract preserves the >= boundary exactly) and difference
        # them with fast DVE subtracts: xb[t] = SG_t - SG_{t+1} in {0,2}.
        # The x2 scale is compensated by halving layer 0's B and D on host.
        emb4 = ctx.enter_context(tc.tile_pool(name="emb4", bufs=1))
        EMBC = emb4.tile([128, KC * TOKPC], F32, tag="embc")
        thr_b = []
        for j, tj in enumerate(THR):
            bt_ = const.tile([128, 1], F32, tag=f"thr{j}", name=f"thr{j}")
            nc.vector.memset(bt_[:], -float(tj))
            thr_b.append(bt_)
        sgp = ctx.enter_context(tc.tile_pool(name="sg", bufs=2))
        with tc.tile_pool(name="enc", bufs=2) as enc, \
             tc.tile_pool(name="encp", bufs=2, space="PSUM") as encps:
            ids_s = enc.tile([128, 2], I32, tag="ids")
            for g in range(2):
                nc.sync.dma_start(ids_s[:, g:g + 1], ids_d[g, :, :])
            for g in range(2):
                eg = enc.tile([128, DM], F32, tag="eg")
                nc.gpsimd.indirect_dma_start(
                    out=eg[:], out_offset=None,
                    in_=emb_d[:, :],
                    in_offset=bass.IndirectOffsetOnAxis(ap=ids_s[:, g:g + 1], axis=0),
                )
                for k in range(KC):
                    pt = encps.tile([128, 128], F32, tag="pt")
                    nc.tensor.transpose(pt[:], eg[:, k * 128:(k + 1) * 128], ident[:])
                    nc.scalar.copy(EMBC[:, k * 256 + g * 128:
                                        k * 256 + g * 128 + 128], pt[:])
            # ascending production, interleaved into layer 0's step loop
            # via emit_encode(t) so the in-order DVE queue never makes the
            # recurrence wait behind the whole encode tail
            _enc_state = {}

            def emit_encode(t):
                # produce xb[t]; requires emit_encode called in ascending t
                if t >= T:
                    return
                if t == 0:
                    sg = sgp.tile([128, KC * TOKPC], F32, tag="sg")
                    nc.scalar.activation(sg[:], EMBC[:], ACTF.Sign,
                                         bias=thr_b[0][:], scale=1.0)
                    nc.vector.tensor_scalar(xb[:, 0:KC * 256], sg[:],
                                            -1.0, 1.0, OP.mult, OP.add)
                    _enc_state["sg"] = sg
                elif t < T - 1:
                    sg_new = sgp.tile([128, KC * TOKPC], F32, tag="sg")
                    nc.scalar.activation(sg_new[:], EMBC[:], ACTF.Sign,
                                         bias=thr_b[t][:], scale=1.0)
                    nc.vector.tensor_tensor(
                        xb[:, t * KC * 256:(t + 1) * KC * 256],
                        _enc_state["sg"][:], sg_new[:], OP.subtract)
                    _enc_state["sg"] = sg_new
                else:
                    nc.vector.tensor_scalar(
                        xb[:, (T - 1) * KC * 256:T * KC * 256],
                        _enc_state["sg"][:], 1.0, None, OP.add)

            emit_encode(0)
            emit_encode(1)

        # ---------------- SSM layers ---------------------------------------
        with tc.tile_pool(name="ssmp", bufs=1, space="PSUM") as ssmps, \
             tc.tile_pool(name="par", bufs=2) as par, \
             tc.tile_pool(name="lif", bufs=2) as lif:
            v1ps = ssmps.tile([128, TOKPC], F32, tag="v1")
            # v2 as two (128,512) tiles: pair j holds feature chunks 2j, 2j+1
            v2pr = [ssmps.tile([128, 2 * TOKPC], F32, tag=f"v2p{j}", name=f"v2pr{j}")
                    for j in range(2)]
            tips = ssmps.tile([128, KC * TOKPC], F32, tag="tips")

            Hprev = None
            for layer in range(L):
                def loadp(dram_ap, shape, tag):
                    pt_ = par.tile(list(shape), F32R, tag=tag, name=f"par_{tag}")
                    nc.sync.dma_start(pt_[:], dram_ap)
                    return pt_

                ah = loadp(at_hi_d[layer, :, :], (128, 128), "ah")
                al = loadp(at_lo_d[layer, :, :], (128, 128), "al")
                bh = loadp(bt_hi_d[layer, :, :, :], (128, KC, 128), "bh")
                bl = loadp(bt_lo_d[layer, :, :, :], (128, KC, 128), "bl")
                ch = loadp(ct_hi_d[layer, :, :, :], (128, KC, 128), "ch")
                cl = loadp(ct_lo_d[layer, :, :, :], (128, KC, 128), "cl")
                dh = loadp(dd_hi_d[layer, :, :, :], (128, KC, 128), "dh")
                dl = loadp(dd_lo_d[layer, :, :, :], (128, KC, 128), "dl")

                def emit_mm2_lif2(t, H_t, xs_t, layer_):
                    # output update accumulation (v2, per chunk) + LIF2
                    for k in range(KC):
                        vsl = v2pr[k // 2][:, (k % 2) * TOKPC:(k % 2 + 1) * TOKPC]
                        mm2 = [(ch[:, k, :], H_t[:]), (cl[:, k, :], H_t[:]),
                               (dh[:, k, :], xs_t[k]), (dl[:, k, :], xs_t[k])]
                        for i, (lhsT, rhs) in enumerate(mm2):
                            nc.tensor.matmul(vsl, lhsT, rhs,
                                             start=(t == 0 and i == 0 and k % 2 == 0),
                                             stop=(i == len(mm2) - 1),
                                             skip_group_check=True)
                    # LIF2: sg on ACT (psum read); bank-0 mask derived on DVE,
                    # bank-1 mask on GPSIMD, so the two 3-hop chains overlap;
                    # v2 *= mask and the spike write (f32r) stay on DVE
                    m2c = lif.tile([128, 2 * 2 * TOKPC], F32, tag="m2c")
                    for j in range(2):
                        sg2 = lif.tile([128, 2 * TOKPC], F32, tag=f"sg2_{j}",
                                       name=f"sg2_{j}")
                        nc.scalar.activation(sg2[:], v2pr[j][:], ACTF.Sign,
                                             bias=neg2[:], scale=1.0)
                        eng = nc.vector if j == 0 else nc.gpsimd
                        eng.tensor_scalar(m2c[:, j * 512:(j + 1) * 512],
                                          sg2[:], -0.25, 0.25,
                                          OP.mult, OP.add)
                        nc.vector.tensor_tensor(v2pr[j][:], v2pr[j][:],
                                                m2c[:, j * 512:(j + 1) * 512],
                                                OP.mult)
                    xsl = xb[:, t * KC * 256:(t + 1) * KC * 256]
                    nc.vector.tensor_scalar(xsl, m2c[:], -2.0, 1.0,
                                            OP.mult, OP.add)
                    if layer_ == L - 1:
                        # time-integration on the PE: tips += I @ X[t]
                        # (reads xsl AFTER the spike overwrite above)
                        for k in range(KC):
                            nc.tensor.matmul(
                                tips[:, k * TOKPC:(k + 1) * TOKPC],
                                ident_r[:], xs_t[k],
                                start=(t == 0 and k % 2 == 0),
                                stop=(t == T - 1),
                                skip_group_check=True)

                prev = None  # (t, H, xs) pending MM2+LIF2 (1-step software skew)
                for t in range(T):
                    if layer == 0:
                        emit_encode(t + 2)
                    xs = [xb[:, (t * KC + k) * 256:(t * KC + k) * 256 + 256]
                          for k in range(KC)]
                    # ---- state update accumulation (v1) ----
                    mm1 = []
                    if t > 0:
                        mm1 += [(ah[:], Hprev[:]), (al[:], Hprev[:])]
                    for k in range(KC):
                        mm1 += [(bh[:, k, :], xs[k]), (bl[:, k, :], xs[k])]
                    for i, (lhsT, rhs) in enumerate(mm1):
                        nc.tensor.matmul(v1ps[:], lhsT, rhs,
                                         start=(t == 0 and i == 0),
                                         stop=(i == len(mm1) - 1),
                                         skip_group_check=True)
                    # ---- LIF1: spike H straight from PSUM on DVE (f32r),
                    #      m1 derived off-path on GPSIMD ----
                    H = lif.tile([128, TOKPC], F32R, tag="H", bufs=3)
                    nc.vector.tensor_scalar(H[:], v1ps[:], 2.0, None, OP.is_ge)
                    m1 = lif.tile([128, TOKPC], F32, tag="m1")
                    nc.gpsimd.tensor_scalar(m1[:], H[:].bitcast(F32), -0.5, 0.5,
                                            OP.mult, OP.add)
                    nc.vector.tensor_tensor(v1ps[:], v1ps[:], m1[:], OP.mult)
                    # ---- previous step's output-side work (keeps PE fed) ----
                    if prev is not None:
                        emit_mm2_lif2(*prev, layer)
                    prev = (t, H, xs)
                    Hprev = H
                emit_mm2_lif2(*prev, layer)

            # time-integrated spike counts -> bf16 (exact integers 0..20;
            # the 1/T scale is folded into Wp on the host)
            for j in range(2):
                nc.scalar.activation(ti_bf[:, j * 512:(j + 1) * 512],
                                     tips[:, j * 512:(j + 1) * 512],
                                     ACTF.Copy, bias=0.0, scale=1.0)

        # ---------------- projection: own 256 tokens x full vocab ----------
        # weight-group DMA triggers are emitted one group AHEAD of the
        # consuming matmuls: the SP engine processes (and blocks on) DMA
        # triggers in order, so output DMAs must not sit in front of the
        # next group's prefetch. Output goes PSUM -> DRAM directly (f32).
        with tc.tile_pool(name="prjp", bufs=2, space="PSUM") as prjps:
            def fetch_w(g):
                wts = []
                for k in range(KC):
                    wt = prw.tile([128, NVW], BF16, tag=f"wt{k}", name=f"wt{k}")
                    eng = nc.sync if k < 2 else nc.scalar
                    eng.dma_start(wt[:], wpt_d[k * 128:(k + 1) * 128,
                                               g * NVW:(g + 1) * NVW])
                    wts.append(wt)
                return wts

            wts_next = fetch_w(0)
            for g in range(NVG):
                wts = wts_next
                if g + 1 < NVG:
                    wts_next = fetch_w(g + 1)
                for m in range(TOKPC // 128):
                    pos = [prjps.tile([128, NV], F32, tag=f"po{nv}",
                                      name=f"po{nv}") for nv in range(NVW // NV)]
                    for k in range(KC):
                        lh = ti_bf[:, k * 256 + m * 128: k * 256 + m * 128 + 128]
                        for nv in range(NVW // NV):
                            nc.tensor.matmul(pos[nv][:], lh,
                                             wts[k][:, nv * NV:(nv + 1) * NV],
                                             start=(k == 0), stop=(k == KC - 1),
                                             skip_group_check=True)
                    osb = osbp.tile([128, NVW], BF16, tag="osb")
                    for nv in range(NVW // NV):
                        nc.scalar.activation(osb[:, nv * NV:(nv + 1) * NV],
                                             pos[nv][:], ACTF.Copy,
                                             bias=0.0, scale=1.0)
                    nc.sync.dma_start(out_d[m * 128:(m + 1) * 128,
                                            g * NVW:(g + 1) * NVW], osb[:])

    nc.compile()
    return nc


_NC_CACHE = {}
_last_in_maps = None


def _get_nc():
    if "nc" not in _NC_CACHE:
        _NC_CACHE["nc"] = _build_nc()
    return _NC_CACHE["nc"]


def kernel(input_ids, emb_table, A, B, C, D, Wp, bp):
    input_ids = np.asarray(input_ids)
    emb_table = np.ascontiguousarray(np.asarray(emb_table), dtype=np.float32)
    A = np.asarray(A, dtype=np.float32)
    B = np.asarray(B, dtype=np.float32)
    C = np.asarray(C, dtype=np.float32)
    D = np.asarray(D, dtype=np.float32)
    Wp = np.asarray(Wp, dtype=np.float32)
    bp = np.asarray(bp, dtype=np.float32)

    ids_flat = input_ids.reshape(-1).astype(np.int32)          # (2048,)

    at = np.ascontiguousarray(A.transpose(0, 2, 1))            # (L,128,128)
    at_hi, at_lo = _hilo(at)
    Bh = B.copy()
    Bh[0] *= np.float32(0.5)   # layer-0 spikes arrive x2 scaled from encode
    bt = np.ascontiguousarray(
        Bh.transpose(2, 0, 1).reshape(KC, 128, L, DS).transpose(2, 1, 0, 3))
    # bt[l,p,k,m] = B[l, m, k*128+p]
    bt_hi, bt_lo = _hilo(bt)
    ct = np.ascontiguousarray(C.transpose(0, 2, 1).reshape(L, 128, KC, 128))
    # ct[l,p,mc,m] = C[l, mc*128+m, p]
    ct_hi, ct_lo = _hilo(ct)
    Dh = D.copy()
    Dh[0] *= np.float32(0.5)   # layer-0 spikes arrive x2 scaled from encode
    dc = Dh.reshape(L, KC, 128)                                # (L,KC,128)
    dc_hi, dc_lo = _hilo(dc)
    eye = np.eye(128, dtype=np.float32)
    # diag matrices for the D (elementwise) term, built on host
    dd_hi = np.ascontiguousarray(
        (eye[None, None, :, :] * dc_hi[:, :, None, :]).transpose(0, 2, 1, 3))
    dd_lo = np.ascontiguousarray(
        (eye[None, None, :, :] * dc_lo[:, :, None, :]).transpose(0, 2, 1, 3))
    # dd[l, p, k, m] = D[l, k*128+p] if p == m else 0

    wpt = np.ascontiguousarray(Wp.T) * np.float32(1.0 / T)     # (512, 32000) f32
    wpt_bf = wpt.astype(ml_dtypes.bfloat16)

    nc = _get_nc()
    in_maps = []
    for c in range(NCORES):
        ids_c = ids_flat[c * TOKPC:(c + 1) * TOKPC].reshape(2, 128, 1)
        in_maps.append({
            "ids": np.ascontiguousarray(ids_c),
            "emb": emb_table,
            "at_hi": at_hi, "at_lo": at_lo,
            "bt_hi": bt_hi, "bt_lo": bt_lo,
            "ct_hi": ct_hi, "ct_lo": ct_lo,
            "dd_hi": dd_hi, "dd_lo": dd_lo,
            "wpt": wpt_bf,
        })

    global _last_in_maps
    _last_in_maps = in_maps
    res = run_bass_kernel_spmd(nc, in_maps, core_ids=list(range(NCORES)))
    outs = [res.results[c]["out"].astype(np.float32) for c in range(NCORES)]
    full = np.concatenate(outs, axis=0)                        # (2048, 32000)
    full += bp[None, :]
    return full.reshape(BATCH, SEQ, VOC).astype(np.float32)


# revision 19
# speedup vs baseline: 1.2323x; 1.0023x over previous
"""Trainium2 Bass kernel for nn_BreakthroughSNN (spiking SSM LM).

Strategy (8 NeuronCores, SPMD single NEFF, fully independent cores):
  - Data-parallel SSM: 2048 tokens (B*S) sharded 256/core. Per core, the
    4-layer x 20-step LIF recurrence runs with persistent membrane
    potentials held in PSUM.
  - All SSM matmuls are fp32r hi/lo pairs (host-split so the device's
    fp32r rounding is exact) -> full fp32-grade precision at 1 cyc/row.
  - Temporal encoding via host-precomputed exact fp32 sigmoid-boundary
    thresholds: per-chunk threshold-count index built with fused
    is_ge/add chains on DVE; the one-hot spike planes are materialized
    just-in-time inside layer 0's step loop (is_equal per step).
  - LIF elementwise work spread across ACT (Sign from PSUM, spike
    writes), GPSIMD (mask affine derivations in SBUF) and DVE (PSUM
    multiply updates) so no single engine gates the recurrence.
  - Projection: token-sharded - each core projects its OWN 256 tokens
    against the FULL 32000 vocab. Wp is streamed as bf16 (pre-scaled by
    1/T on host so the time-integrated spike counts stay integer-exact),
    double-buffered so DMA overlaps the matmuls. No collective at all.
    Output is written bf16 (post-chaos linear op) and upcast + bias on
    host.
"""

import numpy as np
import ml_dtypes
from contextlib import ExitStack

import concourse.bass as bass
import concourse.mybir as mybir
import concourse.tile as tile
from concourse import bacc
from concourse.bass_utils import run_bass_kernel_spmd
from concourse.masks import make_identity

F32 = mybir.dt.float32
F32R = mybir.dt.float32r
BF16 = mybir.dt.bfloat16
I32 = mybir.dt.int32
OP = mybir.AluOpType
ACTF = mybir.ActivationFunctionType

NCORES = 8
TOKPC = 256          # tokens per core
BATCH, SEQ = 4, 512
DM, DS = 512, 128
T, L = 20, 4
VOC = 32000
KC = DM // 128       # 4 feature chunks
NVW = 2000           # vocab cols per proj weight tile (4 psum banks of 500)
NVG = VOC // NVW     # 16 vocab groups
NV = 500             # one PSUM bank of fp32


def _hilo(x):
    x = np.ascontiguousarray(x, dtype=np.float32)
    u = x.view(np.uint32)
    hi = (u & np.uint32(0xFFFFF000)).view(np.float32).copy()  # keep 11 mantissa bits
    lo = (x - hi).astype(np.float32)
    return hi, lo


def _f2key(x):
    u = int(np.array(x, dtype=np.float32).view(np.uint32))
    return (u ^ 0x80000000) if u < 0x80000000 else (0xFFFFFFFF - u)


def _key2f(k):
    u = (k ^ 0x80000000) if k >= 0x80000000 else (0xFFFFFFFF - k)
    return np.array([u], dtype=np.uint32).view(np.float32)[0]


def _g32(x):
    # replicate reference fp32 pipeline: floor happens on this value
    x = np.float32(x)
    s = np.float32(1.0) / (np.float32(1.0) + np.float32(np.exp(np.float32(-x))))
    return np.float32(s * np.float32(19.0))


def _thresholds():
    """T_k = smallest fp32 x with g32(x) >= k, k=1..19 (g32 monotone)."""
    ts = []
    for k in range(1, 20):
        lo_k = _f2key(np.float32(-30.0))
        hi_k = _f2key(np.float32(30.0))
        assert _g32(_key2f(hi_k)) >= k and _g32(_key2f(lo_k)) < k
        while hi_k - lo_k > 1:
            mid = (lo_k + hi_k) // 2
            if _g32(_key2f(mid)) >= k:
                hi_k = mid
            else:
                lo_k = mid
        ts.append(float(_key2f(hi_k)))
    return ts


def _build_nc():
    nc = bacc.Bacc("TRN2", target_bir_lowering=False, debug=False, num_devices=NCORES)

    ids_d = nc.dram_tensor("ids", [2, 128, 1], I32, kind="ExternalInput")
    emb_d = nc.dram_tensor("emb", [VOC, DM], F32, kind="ExternalInput")
    at_hi_d = nc.dram_tensor("at_hi", [L, 128, 128], F32R, kind="ExternalInput")
    at_lo_d = nc.dram_tensor("at_lo", [L, 128, 128], F32R, kind="ExternalInput")
    bt_hi_d = nc.dram_tensor("bt_hi", [L, 128, KC, 128], F32R, kind="ExternalInput")
    bt_lo_d = nc.dram_tensor("bt_lo", [L, 128, KC, 128], F32R, kind="ExternalInput")
    ct_hi_d = nc.dram_tensor("ct_hi", [L, 128, KC, 128], F32R, kind="ExternalInput")
    ct_lo_d = nc.dram_tensor("ct_lo", [L, 128, KC, 128], F32R, kind="ExternalInput")
    dd_hi_d = nc.dram_tensor("dd_hi", [L, 128, KC, 128], F32R, kind="ExternalInput")
    dd_lo_d = nc.dram_tensor("dd_lo", [L, 128, KC, 128], F32R, kind="ExternalInput")
    wpt_d = nc.dram_tensor("wpt", [DM, VOC], BF16, kind="ExternalInput")
    out_d = nc.dram_tensor("out", [TOKPC, VOC], BF16, kind="ExternalOutput")

    THR = _thresholds()

    with tile.TileContext(nc) as tc, ExitStack() as ctx:
        const = ctx.enter_context(tc.tile_pool(name="const", bufs=1))
        ident = const.tile([128, 128], F32)
        make_identity(nc, ident[:])
        ident_r = const.tile([128, 128], F32R)
        nc.vector.tensor_copy(ident_r[:], ident[:])
        neg2 = const.tile([128, 1], F32)
        nc.vector.memset(neg2[:], -2.0)

        xb_pool = ctx.enter_context(tc.tile_pool(name="xb", bufs=1))
        xb = xb_pool.tile([128, T * KC * 256], F32R)
        tip = ctx.enter_context(tc.tile_pool(name="ti", bufs=1))
        ti_bf = tip.tile([128, KC * 256], BF16, tag="tibf")
        # projection weight pool created BEFORE the SSM pools so its SBUF
        # region doesn't overlap theirs -> the first weight-group DMAs can
        # prefetch during the SSM phase
        prw = ctx.enter_context(tc.tile_pool(name="prw", bufs=2))
        osbp = ctx.enter_context(tc.tile_pool(name="osb", bufs=3))

        # ---------------- encode: gather + transpose + sign one-hot --------
        # Compare ALU ops are pathologically slow on DVE/GPSIMD; instead
        # build step functions SG_t = Sign(EMB - T_t) in {-1,1} on ACT
        # (IEEE subtract preserves the >= boundary exactly) and difference
        # them with fast DVE subtracts: xb[t] = SG_t - SG_{t+1} in {0,2}.
        # The x2 scale is compensated by halving layer 0's B and D on host.
        emb4 = ctx.enter_context(tc.tile_pool(name="emb4", bufs=1))
        EMBC = emb4.tile([128, KC * TOKPC], F32, tag="embc")
        thr_b = []
        for j, tj in enumerate(THR):
            bt_ = const.tile([128, 1], F32, tag=f"thr{j}", name=f"thr{j}")
            nc.vector.memset(bt_[:], -float(tj))
            thr_b.append(bt_)
        sgp = ctx.enter_context(tc.tile_pool(name="sg", bufs=2))
        with tc.tile_pool(name="enc", bufs=2) as enc, \
             tc.tile_pool(name="encp", bufs=2, space="PSUM") as encps:
            ids_s = enc.tile([128, 2], I32, tag="ids")
            for g in range(2):
                nc.sync.dma_start(ids_s[:, g:g + 1], ids_d[g, :, :])
            for g in range(2):
                eg = enc.tile([128, DM], F32, tag="eg")
                nc.gpsimd.indirect_dma_start(
                    out=eg[:], out_offset=None,
                    in_=emb_d[:, :],
                    in_offset=bass.IndirectOffsetOnAxis(ap=ids_s[:, g:g + 1], axis=0),
                )
                for k in range(KC):
                    pt = encps.tile([128, 128], F32, tag="pt")
                    nc.tensor.transpose(pt[:], eg[:, k * 128:(k + 1) * 128], ident[:])
                    nc.scalar.copy(EMBC[:, k * 256 + g * 128:
                                        k * 256 + g * 128 + 128], pt[:])
            # ascending production, interleaved into layer 0's step loop
            # via emit_encode(t) so the in-order DVE queue never makes the
            # recurrence wait behind the whole encode tail
            _enc_state = {}

            def emit_encode(t):
                # produce xb[t]; requires emit_encode called in ascending t
                if t >= T:
                    return
                if t == 0:
                    sg = sgp.tile([128, KC * TOKPC], F32, tag="sg")
                    nc.scalar.activation(sg[:], EMBC[:], ACTF.Sign,
                                         bias=thr_b[0][:], scale=1.0)
                    nc.vector.tensor_scalar(xb[:, 0:KC * 256], sg[:],
                                            -1.0, 1.0, OP.mult, OP.add)
                    _enc_state["sg"] = sg
                elif t < T - 1:
                    sg_new = sgp.tile([128, KC * TOKPC], F32, tag="sg")
                    nc.scalar.activation(sg_new[:], EMBC[:], ACTF.Sign,
                                         bias=thr_b[t][:], scale=1.0)
                    # split halves across DVE/GPSIMD: layer 0 is DVE-bound
                    base = t * KC * 256
                    nc.vector.tensor_tensor(
                        xb[:, base:base + 512],
                        _enc_state["sg"][:, 0:512], sg_new[:, 0:512],
                        OP.subtract)
                    nc.gpsimd.tensor_tensor(
                        xb[:, base + 512:base + 1024],
                        _enc_state["sg"][:, 512:1024], sg_new[:, 512:1024],
                        OP.subtract)
                    _enc_state["sg"] = sg_new
                else:
                    nc.vector.tensor_scalar(
                        xb[:, (T - 1) * KC * 256:T * KC * 256],
                        _enc_state["sg"][:], 1.0, None, OP.add)

            emit_encode(0)
            emit_encode(1)

        # ---------------- SSM layers ---------------------------------------
        with tc.tile_pool(name="ssmp", bufs=1, space="PSUM") as ssmps, \
             tc.tile_pool(name="par", bufs=2) as par, \
             tc.tile_pool(name="lif", bufs=2) as lif:
            v1ps = ssmps.tile([128, TOKPC], F32, tag="v1")
            # v2 as two (128,512) tiles: pair j holds feature chunks 2j, 2j+1
            v2pr = [ssmps.tile([128, 2 * TOKPC], F32, tag=f"v2p{j}", name=f"v2pr{j}")
                    for j in range(2)]
            tips = ssmps.tile([128, KC * TOKPC], F32, tag="tips")

            Hprev = None
            for layer in range(L):
                def loadp(dram_ap, shape, tag):
                    pt_ = par.tile(list(shape), F32R, tag=tag, name=f"par_{tag}")
                    nc.sync.dma_start(pt_[:], dram_ap)
                    return pt_

                ah = loadp(at_hi_d[layer, :, :], (128, 128), "ah")
                al = loadp(at_lo_d[layer, :, :], (128, 128), "al")
                bh = loadp(bt_hi_d[layer, :, :, :], (128, KC, 128), "bh")
                bl = loadp(bt_lo_d[layer, :, :, :], (128, KC, 128), "bl")
                ch = loadp(ct_hi_d[layer, :, :, :], (128, KC, 128), "ch")
                cl = loadp(ct_lo_d[layer, :, :, :], (128, KC, 128), "cl")
                dh = loadp(dd_hi_d[layer, :, :, :], (128, KC, 128), "dh")
                dl = loadp(dd_lo_d[layer, :, :, :], (128, KC, 128), "dl")

                def emit_mm2_lif2(t, H_t, xs_t, layer_):
                    # output update accumulation (v2, per chunk) + LIF2
                    for k in range(KC):
                        vsl = v2pr[k // 2][:, (k % 2) * TOKPC:(k % 2 + 1) * TOKPC]
                        mm2 = [(ch[:, k, :], H_t[:]), (cl[:, k, :], H_t[:]),
                               (dh[:, k, :], xs_t[k]), (dl[:, k, :], xs_t[k])]
                        for i, (lhsT, rhs) in enumerate(mm2):
                            nc.tensor.matmul(vsl, lhsT, rhs,
                                             start=(t == 0 and i == 0 and k % 2 == 0),
                                             stop=(i == len(mm2) - 1),
                                             skip_group_check=True)
                    # LIF2: sg on ACT (psum read); bank-0 mask derived on DVE,
                    # bank-1 mask on GPSIMD, so the two 3-hop chains overlap;
                    # v2 *= mask and the spike write (f32r) stay on DVE
                    m2c = lif.tile([128, 2 * 2 * TOKPC], F32, tag="m2c")
                    for j in range(2):
                        sg2 = lif.tile([128, 2 * TOKPC], F32, tag=f"sg2_{j}",
                                       name=f"sg2_{j}")
                        nc.scalar.activation(sg2[:], v2pr[j][:], ACTF.Sign,
                                             bias=neg2[:], scale=1.0)
                        eng = nc.vector if j == 0 else nc.gpsimd
                        eng.tensor_scalar(m2c[:, j * 512:(j + 1) * 512],
                                          sg2[:], -0.25, 0.25,
                                          OP.mult, OP.add)
                        nc.vector.tensor_tensor(v2pr[j][:], v2pr[j][:],
                                                m2c[:, j * 512:(j + 1) * 512],
                                                OP.mult)
                    xsl = xb[:, t * KC * 256:(t + 1) * KC * 256]
                    nc.vector.tensor_scalar(xsl, m2c[:], -2.0, 1.0,
                                            OP.mult, OP.add)
                    if layer_ == L - 1:
                        # time-integration on the PE: tips += I @ X[t]
                        # (reads xsl AFTER the spike overwrite above)
                        for k in range(KC):
                            nc.tensor.matmul(
                                tips[:, k * TOKPC:(k + 1) * TOKPC],
                                ident_r[:], xs_t[k],
                                start=(t == 0 and k % 2 == 0),
                                stop=(t == T - 1),
                                skip_group_check=True)

                prev = None  # (t, H, xs) pending MM2+LIF2 (1-step software skew)
                for t in range(T):
                    if layer == 0:
                        emit_encode(t + 2)
                    xs = [xb[:, (t * KC + k) * 256:(t * KC + k) * 256 + 256]
                          for k in range(KC)]
                    # ---- state update accumulation (v1) ----
                    mm1 = []
                    if t > 0:
                        mm1 += [(ah[:], Hprev[:]), (al[:], Hprev[:])]
                    for k in range(KC):
                        mm1 += [(bh[:, k, :], xs[k]), (bl[:, k, :], xs[k])]
                    for i, (lhsT, rhs) in enumerate(mm1):
                        nc.tensor.matmul(v1ps[:], lhsT, rhs,
                                         start=(t == 0 and i == 0),
                                         stop=(i == len(mm1) - 1),
                                         skip_group_check=True)
                    # ---- LIF1: spike H straight from PSUM on DVE (f32r),
                    #      m1 derived off-path on GPSIMD ----
                    H = lif.tile([128, TOKPC], F32R, tag="H", bufs=3)
                    nc.vector.tensor_scalar(H[:], v1ps[:], 2.0, None, OP.is_ge)
                    m1 = lif.tile([128, TOKPC], F32, tag="m1")
                    nc.gpsimd.tensor_scalar(m1[:], H[:].bitcast(F32), -0.5, 0.5,
                                            OP.mult, OP.add)
                    nc.vector.tensor_tensor(v1ps[:], v1ps[:], m1[:], OP.mult)
                    # ---- previous step's output-side work (keeps PE fed) ----
                    if prev is not None:
                        emit_mm2_lif2(*prev, layer)
                    prev = (t, H, xs)
                    Hprev = H
                emit_mm2_lif2(*prev, layer)

            # time-integrated spike counts -> bf16 (exact integers 0..20;
            # the 1/T scale is folded into Wp on the host)
            for j in range(2):
                nc.scalar.activation(ti_bf[:, j * 512:(j + 1) * 512],
                                     tips[:, j * 512:(j + 1) * 512],
                                     ACTF.Copy, bias=0.0, scale=1.0)

        # ---------------- projection: own 256 tokens x full vocab ----------
        # weight-group DMA triggers are emitted one group AHEAD of the
        # consuming matmuls: the SP engine processes (and blocks on) DMA
        # triggers in order, so output DMAs must not sit in front of the
        # next group's prefetch. Output goes PSUM -> DRAM directly (f32).
        with tc.tile_pool(name="prjp", bufs=2, space="PSUM") as prjps:
            def fetch_w(g):
                wts = []
                for k in range(KC):
                    wt = prw.tile([128, NVW], BF16, tag=f"wt{k}", name=f"wt{k}")
                    eng = nc.sync if k < 2 else nc.scalar
                    eng.dma_start(wt[:], wpt_d[k * 128:(k + 1) * 128,
                                               g * NVW:(g + 1) * NVW])
                    wts.append(wt)
                return wts

            wts_next = fetch_w(0)
            for g in range(NVG):
                wts = wts_next
                if g + 1 < NVG:
                    wts_next = fetch_w(g + 1)
                for m in range(TOKPC // 128):
                    pos = [prjps.tile([128, NV], F32, tag=f"po{nv}",
                                      name=f"po{nv}") for nv in range(NVW // NV)]
                    for k in range(KC):
                        lh = ti_bf[:, k * 256 + m * 128: k * 256 + m * 128 + 128]
                        for nv in range(NVW // NV):
                            nc.tensor.matmul(pos[nv][:], lh,
                                             wts[k][:, nv * NV:(nv + 1) * NV],
                                             start=(k == 0), stop=(k == KC - 1),
                                             skip_group_check=True)
                    osb = osbp.tile([128, NVW], BF16, tag="osb")
                    for nv in range(NVW // NV):
                        nc.scalar.activation(osb[:, nv * NV:(nv + 1) * NV],
                                             pos[nv][:], ACTF.Copy,
                                             bias=0.0, scale=1.0)
                    nc.sync.dma_start(out_d[m * 128:(m + 1) * 128,
                                            g * NVW:(g + 1) * NVW], osb[:])

    nc.compile()
    return nc


_NC_CACHE = {}
_last_in_maps = None


def _get_nc():
    if "nc" not in _NC_CACHE:
        _NC_CACHE["nc"] = _build_nc()
    return _NC_CACHE["nc"]


def kernel(input_ids, emb_table, A, B, C, D, Wp, bp):
    input_ids = np.asarray(input_ids)
    emb_table = np.ascontiguousarray(np.asarray(emb_table), dtype=np.float32)
    A = np.asarray(A, dtype=np.float32)
    B = np.asarray(B, dtype=np.float32)
    C = np.asarray(C, dtype=np.float32)
    D = np.asarray(D, dtype=np.float32)
    Wp = np.asarray(Wp, dtype=np.float32)
    bp = np.asarray(bp, dtype=np.float32)

    ids_flat = input_ids.reshape(-1).astype(np.int32)          # (2048,)

    at = np.ascontiguousarray(A.transpose(0, 2, 1))            # (L,128,128)
    at_hi, at_lo = _hilo(at)
    Bh = B.copy()
    Bh[0] *= np.float32(0.5)   # layer-0 spikes arrive x2 scaled from encode
    bt = np.ascontiguousarray(
        Bh.transpose(2, 0, 1).reshape(KC, 128, L, DS).transpose(2, 1, 0, 3))
    # bt[l,p,k,m] = B[l, m, k*128+p]
    bt_hi, bt_lo = _hilo(bt)
    ct = np.ascontiguousarray(C.transpose(0, 2, 1).reshape(L, 128, KC, 128))
    # ct[l,p,mc,m] = C[l, mc*128+m, p]
    ct_hi, ct_lo = _hilo(ct)
    Dh = D.copy()
    Dh[0] *= np.float32(0.5)   # layer-0 spikes arrive x2 scaled from encode
    dc = Dh.reshape(L, KC, 128)                                # (L,KC,128)
    dc_hi, dc_lo = _hilo(dc)
    eye = np.eye(128, dtype=np.float32)
    # diag matrices for the D (elementwise) term, built on host
    dd_hi = np.ascontiguousarray(
        (eye[None, None, :, :] * dc_hi[:, :, None, :]).transpose(0, 2, 1, 3))
    dd_lo = np.ascontiguousarray(
        (eye[None, None, :, :] * dc_lo[:, :, None, :]).transpose(0, 2, 1, 3))
    # dd[l, p, k, m] = D[l, k*128+p] if p == m else 0

    wpt = np.ascontiguousarray(Wp.T) * np.float32(1.0 / T)     # (512, 32000) f32
    wpt_bf = wpt.astype(ml_dtypes.bfloat16)

    nc = _get_nc()
    in_maps = []
    for c in range(NCORES):
        ids_c = ids_flat[c * TOKPC:(c + 1) * TOKPC].reshape(2, 128, 1)
        in_maps.append({
            "ids": np.ascontiguousarray(ids_c),
            "emb": emb_table,
            "at_hi": at_hi, "at_lo": at_lo,
            "bt_hi": bt_hi, "bt_lo": bt_lo,
            "ct_hi": ct_hi, "ct_lo": ct_lo,
            "dd_hi": dd_hi, "dd_lo": dd_lo,
            "wpt": wpt_bf,
        })

    global _last_in_maps
    _last_in_maps = in_maps
    res = run_bass_kernel_spmd(nc, in_maps, core_ids=list(range(NCORES)))
    outs = [res.results[c]["out"].astype(np.float32) for c in range(NCORES)]
    full = np.concatenate(outs, axis=0)                        # (2048, 32000)
    full += bp[None, :]
    return full.reshape(BATCH, SEQ, VOC).astype(np.float32)
